# revision 10
# baseline (speedup 1.0000x reference)
"""Fused MoE expert-parallel MLP kernel for Trainium2 (8 NeuronCores).

Problem: y[b, e*T+t, :] = gelu(x[b, e*T+t, :] @ W1[e] + b1[e]) @ W2[e] + b2[e]
with B=4, E=8, T=2048, D=1024, F=4096.

Sharding: expert parallelism — core e runs expert e on its 8192 tokens.
No cross-core communication needed; host slices x per expert and
concatenates the per-core outputs.

Per-core kernel (all matmuls bf16, accumulation fp32 in PSUM):
  x is fed pre-transposed as xT [D, NTOK] so the contraction dim D lands
  on SBUF partitions. For each 512-token tile:
    phase 1: for each of 32 F-slices (128 wide): h_psum[f,t] = sum_d
             W1-block.T @ xT-block (8 matmuls, N=512), then one ScalarE
             activation applies bias b1 (per-partition) + exact Gelu and
             writes h to SBUF as bf16 [128f x 512t].
    phase 2: for each 128-token block: y_psum[t, 0:1024] accumulates
             h-block.T @ W2-slice over the 32 F-slices; DVE adds the
             (partition-broadcast) b2 while evacuating PSUM to SBUF and
             the result DMAs out natively as y [NTOK, D] fp32.

DMA order is tuned so the first matmul's dependencies (x tile 0 +
the first quarter of W1) land in a few microseconds instead of waiting
for all 16.8 MB of weights.
"""

import numpy as np
import ml_dtypes

B, E, T, D, F = 4, 8, 2048, 1024, 4096
NTOK = B * T          # tokens per expert/core
TT = 512              # token tile
NT = NTOK // TT       # 16 token tiles
DS = D // 128         # 8 d-slices
FS = F // 128         # 32 f-slices
TB = TT // 128        # 4 token blocks per tile
W1C = 16              # W1 DMA chunks (along F)

_CACHE = {}
LAST_RESULT = None


def _build_nc():
    import concourse.bass as bass
    import concourse.mybir as mybir
    import concourse.tile as tile
    from concourse import bacc
    from contextlib import ExitStack

    bf16 = mybir.dt.bfloat16
    f32 = mybir.dt.float32
    AF = mybir.ActivationFunctionType

    nc = bacc.Bacc("TRN2", target_bir_lowering=False, debug=False, num_devices=E)

    xT = nc.declare_dram_parameter("xT", [D, NTOK], bf16, isOutput=False)
    W1 = nc.declare_dram_parameter("W1", [D, F], bf16, isOutput=False)
    b1 = nc.declare_dram_parameter("b1", [128, FS], f32, isOutput=False)
    W2 = nc.declare_dram_parameter("W2", [F, D], bf16, isOutput=False)
    b2 = nc.declare_dram_parameter("b2", [1, D], f32, isOutput=False)
    y = nc.declare_dram_parameter("y", [NTOK, D], f32, isOutput=True)

    xT_t = xT[:, :].rearrange("(s p) n -> p s n", p=128)    # [128, DS, NTOK]
    W1_t = W1[:, :].rearrange("(s p) f -> p s f", p=128)    # [128, DS, F]
    W2_t = W2[:, :].rearrange("(s p) d -> p s d", p=128)    # [128, FS, D]

    # b2 replicated across all 128 partitions via broadcast DMA
    b2_bcast = bass.AP(
        tensor=b2[:, :].tensor,
        offset=0,
        ap=[[0, 128], [1, D]],
    )

    with tile.TileContext(nc) as tc, ExitStack() as ctx:
        const = ctx.enter_context(tc.tile_pool(name="const", bufs=1))
        wpool = ctx.enter_context(tc.tile_pool(name="weights", bufs=1))
        xpool = ctx.enter_context(tc.tile_pool(name="x", bufs=2))
        hpool = ctx.enter_context(tc.tile_pool(name="h", bufs=1))
        ypool = ctx.enter_context(tc.tile_pool(name="y", bufs=3))
        hps_pool = ctx.enter_context(tc.tile_pool(name="hps", bufs=3, space="PSUM"))
        yps_pool = ctx.enter_context(tc.tile_pool(name="yps", bufs=2, space="PSUM"))

        # --- warm up the ScalarE Gelu LUT so the first real gelu doesn't
        # pay the ACT_TABLE_LOAD on the critical path ---
        warm = const.tile([128, 1], f32)
        nc.vector.memset(warm, 0.0)
        warm_out = const.tile([128, 1], f32)
        nc.scalar.activation(warm_out, warm, AF.Gelu, bias=0.0)

        # --- startup DMAs, ordered so the first matmuls unblock ASAP.
        # The first x tile and W1 chunk are split into many small DMAs so
        # they fan out across DMA queues/engines in parallel. ---
        b1s = const.tile([128, FS], f32)
        nc.sync.dma_start(out=b1s, in_=b1[:, :])

        # spread the startup-critical loads across several engines' DGE
        # queues so they move in parallel instead of serializing on one
        dma_engines = [nc.sync, nc.scalar, nc.gpsimd]
        xts0 = xpool.tile([128, DS, TT], bf16, name="xts")
        for s in range(DS):
            dma_engines[s % 3].dma_start(out=xts0[:, s, :], in_=xT_t[:, s, 0:TT])

        w1s = wpool.tile([128, DS, F], bf16)
        FCH = F // W1C
        for c in range(W1C):
            dma_engines[c % 3].dma_start(
                out=w1s[:, :, c * FCH:(c + 1) * FCH],
                in_=W1_t[:, :, c * FCH:(c + 1) * FCH],
            )

        # --- warm the PE HAM clock gate with dummy matmuls while the
        # startup DMAs are in flight, so real matmuls start at 2.4 GHz ---
        wones = const.tile([1, 512], bf16)
        nc.vector.memset(wones, 1.0)
        warm_ps = ctx.enter_context(
            tc.tile_pool(name="warmps", bufs=1, space="PSUM")
        ).tile([128, 512], f32)
        for _ in range(12):
            nc.tensor.matmul(
                warm_ps, lhsT=wones[:, 0:128], rhs=wones, start=True, stop=True
            )
        b2s = const.tile([128, D], f32)
        nc.sync.dma_start(out=b2s, in_=b2_bcast)

        w2s = wpool.tile([128, FS, D], bf16)
        nc.sync.dma_start(out=w2s, in_=W2_t)

        for i in range(NT):
            if i == 0:
                xts = xts0
            else:
                xts = xpool.tile([128, DS, TT], bf16, name="xts")
                nc.sync.dma_start(out=xts, in_=xT_t[:, :, i * TT:(i + 1) * TT])

            h = hpool.tile([128, FS, TT], bf16)
            for j in range(FS):
                hps = hps_pool.tile([128, TT], f32)
                for s in range(DS):
                    nc.tensor.matmul(
                        hps,
                        lhsT=w1s[:, s, j * 128:(j + 1) * 128],
                        rhs=xts[:, s, :],
                        start=(s == 0),
                        stop=(s == DS - 1),
                    )
                nc.scalar.activation(
                    h[:, j, :], hps, AF.Gelu, bias=b1s[:, j:j + 1]
                )

            for bb in range(TB):
                yps = yps_pool.tile([128, D], f32)
                for j in range(FS):
                    lh = h[:, j, bb * 128:(bb + 1) * 128]
                    nc.tensor.matmul(
                        yps[:, 0:512], lhsT=lh, rhs=w2s[:, j, 0:512],
                        start=(j == 0), stop=(j == FS - 1),
                    )
                    nc.tensor.matmul(
                        yps[:, 512:1024], lhsT=lh, rhs=w2s[:, j, 512:1024],
                        start=(j == 0), stop=(j == FS - 1),
                    )
                ysb = ypool.tile([128, D], f32)
                nc.vector.tensor_add(ysb, yps, b2s)
                nc.sync.dma_start(
                    out=y[i * TT + bb * 128:i * TT + (bb + 1) * 128, :],
                    in_=ysb,
                )

    nc.compile()
    return nc


def _get_nc():
    if "nc" not in _CACHE:
        _CACHE["nc"] = _build_nc()
    return _CACHE["nc"]


def _ensure_ntff_hook():
    """Register the axon NTFF profile hook (needed only for trace=True)."""
    import sys
    import types

    if "antenv.axon_hooks" in sys.modules:
        return
    mod = types.ModuleType("antenv.axon_hooks")
    _hook = [None]
    mod.set_axon_ntff_profile_hook = lambda h: _hook.__setitem__(0, h)
    mod.get_axon_ntff_profile_hook = lambda: _hook[0]
    sys.modules["antenv.axon_hooks"] = mod
    try:
        import antenv

        antenv.axon_hooks = mod
    except ImportError:
        pass
    try:
        from trn_agent_boot.trn_boot import _ntff_profile_via_ctypes

        mod.set_axon_ntff_profile_hook(
            _ntff_profile_via_ctypes("/opt/axon/libaxon_pjrt.so")
        )
    except Exception:
        pass


def kernel(x, W1, b1, W2, b2, trace=False):
    global LAST_RESULT
    from concourse.bass_utils import run_bass_kernel_spmd

    if trace:
        _ensure_ntff_hook()

    bf16 = ml_dtypes.bfloat16
    x = np.asarray(x, dtype=np.float32)
    W1 = np.asarray(W1, dtype=np.float32)
    b1 = np.asarray(b1, dtype=np.float32)
    W2 = np.asarray(W2, dtype=np.float32)
    b2 = np.asarray(b2, dtype=np.float32)

    xr = x.reshape(B, E, T, D)
    in_maps = []
    for e in range(E):
        xe = np.ascontiguousarray(xr[:, e]).reshape(NTOK, D)
        in_maps.append({
            "xT": xe.T.astype(bf16),                          # [D, NTOK]
            "W1": W1[e].astype(bf16),                         # [D, F]
            "b1": np.ascontiguousarray(b1[e].reshape(FS, 128).T),  # [128, FS]
            "W2": W2[e].astype(bf16),                         # [F, D]
            "b2": b2[e].reshape(1, D).copy(),                 # fp32
        })

    nc = _get_nc()
    res = run_bass_kernel_spmd(nc, in_maps, list(range(E)), trace=trace)
    LAST_RESULT = res

    out = np.empty((B, E * T, D), dtype=np.float32)
    for e in range(E):
        out[:, e * T:(e + 1) * T, :] = res.results[e]["y"].reshape(B, T, D)
    return out


# revision 12
# speedup vs baseline: 1.0070x; 1.0070x over previous
"""Fused MoE expert-parallel MLP kernel for Trainium2 (8 NeuronCores).

Problem: y[b, e*T+t, :] = gelu(x[b, e*T+t, :] @ W1[e] + b1[e]) @ W2[e] + b2[e]
with B=4, E=8, T=2048, D=1024, F=4096.

Sharding: expert parallelism — core e runs expert e on its 8192 tokens.
No cross-core communication needed; host slices x per expert and
concatenates the per-core outputs.

Per-core kernel (all matmuls bf16, accumulation fp32 in PSUM):
  x is fed pre-transposed as xT [D, NTOK] so the contraction dim D lands
  on SBUF partitions. For each 512-token tile:
    phase 1: for each of 32 F-slices (128 wide): h_psum[f,t] = sum_d
             W1-block.T @ xT-block (8 matmuls, N=512), then one ScalarE
             activation applies bias b1 (per-partition) + exact Gelu and
             writes h to SBUF as bf16 [128f x 512t].
    phase 2: for each 128-token block: y_psum[t, 0:1024] accumulates
             h-block.T @ W2-slice over the 32 F-slices; DVE adds the
             (partition-broadcast) b2 while evacuating PSUM to SBUF and
             the result DMAs out natively as y [NTOK, D] fp32.

DMA order is tuned so the first matmul's dependencies (x tile 0 +
the first quarter of W1) land in a few microseconds instead of waiting
for all 16.8 MB of weights.
"""

import numpy as np
import ml_dtypes

B, E, T, D, F = 4, 8, 2048, 1024, 4096
NTOK = B * T          # tokens per expert/core
TT = 512              # token tile
NT = NTOK // TT       # 16 token tiles
DS = D // 128         # 8 d-slices
FS = F // 128         # 32 f-slices
TB = TT // 128        # 4 token blocks per tile
W1C = 16              # W1 DMA chunks (along F)

_CACHE = {}
LAST_RESULT = None


def _build_nc():
    import concourse.bass as bass
    import concourse.mybir as mybir
    import concourse.tile as tile
    from concourse import bacc
    from contextlib import ExitStack

    bf16 = mybir.dt.bfloat16
    f32 = mybir.dt.float32
    AF = mybir.ActivationFunctionType

    nc = bacc.Bacc("TRN2", target_bir_lowering=False, debug=False, num_devices=E)

    xT = nc.declare_dram_parameter("xT", [D, NTOK], bf16, isOutput=False)
    W1 = nc.declare_dram_parameter("W1", [D, F], bf16, isOutput=False)
    b1 = nc.declare_dram_parameter("b1", [128, FS], f32, isOutput=False)
    W2 = nc.declare_dram_parameter("W2", [F, D], bf16, isOutput=False)
    b2 = nc.declare_dram_parameter("b2", [1, D], f32, isOutput=False)
    y = nc.declare_dram_parameter("y", [NTOK, D], f32, isOutput=True)

    xT_t = xT[:, :].rearrange("(s p) n -> p s n", p=128)    # [128, DS, NTOK]
    W1_t = W1[:, :].rearrange("(s p) f -> p s f", p=128)    # [128, DS, F]
    W2_t = W2[:, :].rearrange("(s p) d -> p s d", p=128)    # [128, FS, D]

    # b2 replicated across all 128 partitions via broadcast DMA
    b2_bcast = bass.AP(
        tensor=b2[:, :].tensor,
        offset=0,
        ap=[[0, 128], [1, D]],
    )

    with tile.TileContext(nc) as tc, ExitStack() as ctx:
        const = ctx.enter_context(tc.tile_pool(name="const", bufs=1))
        wpool = ctx.enter_context(tc.tile_pool(name="weights", bufs=1))
        xpool = ctx.enter_context(tc.tile_pool(name="x", bufs=2))
        hpool = ctx.enter_context(tc.tile_pool(name="h", bufs=1))
        ypool = ctx.enter_context(tc.tile_pool(name="y", bufs=3))
        hps_pool = ctx.enter_context(tc.tile_pool(name="hps", bufs=3, space="PSUM"))
        yps_pool = ctx.enter_context(tc.tile_pool(name="yps", bufs=2, space="PSUM"))

        # --- warm up the ScalarE Gelu LUT so the first real gelu doesn't
        # pay the ACT_TABLE_LOAD on the critical path ---
        warm = const.tile([128, 1], f32)
        nc.vector.memset(warm, 0.0)
        warm_out = const.tile([128, 1], f32)
        nc.scalar.activation(warm_out, warm, AF.Gelu, bias=0.0)

        # --- startup DMAs, ordered so the first matmuls unblock ASAP:
        # tiny b1 first, then the first x tile, then W1 in small chunks so
        # phase-1 compute can start while the rest of W1/W2 streams in. ---
        b1s = const.tile([128, FS], f32)
        nc.sync.dma_start(out=b1s, in_=b1[:, :])

        xts0 = xpool.tile([128, DS, TT], bf16, name="xts")
        for s in range(DS):
            nc.sync.dma_start(out=xts0[:, s, :], in_=xT_t[:, s, 0:TT])

        w1s = wpool.tile([128, DS, F], bf16)
        FCH = F // W1C
        for c in range(W1C):
            nc.sync.dma_start(
                out=w1s[:, :, c * FCH:(c + 1) * FCH],
                in_=W1_t[:, :, c * FCH:(c + 1) * FCH],
            )

        # --- warm the PE HAM clock gate with dummy matmuls while the
        # startup DMAs are in flight, so real matmuls start at 2.4 GHz ---
        wones = const.tile([1, 512], bf16)
        nc.vector.memset(wones, 1.0)
        warm_ps = ctx.enter_context(
            tc.tile_pool(name="warmps", bufs=1, space="PSUM")
        ).tile([128, 512], f32)
        for _ in range(12):
            nc.tensor.matmul(
                warm_ps, lhsT=wones[:, 0:128], rhs=wones, start=True, stop=True
            )
        b2s = const.tile([128, D], f32)
        nc.sync.dma_start(out=b2s, in_=b2_bcast)

        w2s = wpool.tile([128, FS, D], bf16)
        nc.sync.dma_start(out=w2s, in_=W2_t)

        for i in range(NT):
            if i == 0:
                xts = xts0
            else:
                xts = xpool.tile([128, DS, TT], bf16, name="xts")
                nc.sync.dma_start(out=xts, in_=xT_t[:, :, i * TT:(i + 1) * TT])

            h = hpool.tile([128, FS, TT], bf16)
            for j in range(FS):
                hps = hps_pool.tile([128, TT], f32)
                for s in range(DS):
                    nc.tensor.matmul(
                        hps,
                        lhsT=w1s[:, s, j * 128:(j + 1) * 128],
                        rhs=xts[:, s, :],
                        start=(s == 0),
                        stop=(s == DS - 1),
                    )
                nc.scalar.activation(
                    h[:, j, :], hps, AF.Gelu, bias=b1s[:, j:j + 1]
                )

            for bb in range(TB):
                yps = yps_pool.tile([128, D], f32)
                for j in range(FS):
                    lh = h[:, j, bb * 128:(bb + 1) * 128]
                    nc.tensor.matmul(
                        yps[:, 0:512], lhsT=lh, rhs=w2s[:, j, 0:512],
                        start=(j == 0), stop=(j == FS - 1),
                    )
                    nc.tensor.matmul(
                        yps[:, 512:1024], lhsT=lh, rhs=w2s[:, j, 512:1024],
                        start=(j == 0), stop=(j == FS - 1),
                    )
                ysb = ypool.tile([128, D], f32)
                nc.vector.tensor_add(ysb, yps, b2s)
                nc.sync.dma_start(
                    out=y[i * TT + bb * 128:i * TT + (bb + 1) * 128, :],
                    in_=ysb,
                )

    nc.compile()
    return nc


def _get_nc():
    if "nc" not in _CACHE:
        _CACHE["nc"] = _build_nc()
    return _CACHE["nc"]


def _ensure_ntff_hook():
    """Register the axon NTFF profile hook (needed only for trace=True)."""
    import sys
    import types

    if "antenv.axon_hooks" in sys.modules:
        return
    mod = types.ModuleType("antenv.axon_hooks")
    _hook = [None]
    mod.set_axon_ntff_profile_hook = lambda h: _hook.__setitem__(0, h)
    mod.get_axon_ntff_profile_hook = lambda: _hook[0]
    sys.modules["antenv.axon_hooks"] = mod
    try:
        import antenv

        antenv.axon_hooks = mod
    except ImportError:
        pass
    try:
        from trn_agent_boot.trn_boot import _ntff_profile_via_ctypes

        mod.set_axon_ntff_profile_hook(
            _ntff_profile_via_ctypes("/opt/axon/libaxon_pjrt.so")
        )
    except Exception:
        pass


def kernel(x, W1, b1, W2, b2, trace=False):
    global LAST_RESULT
    from concourse.bass_utils import run_bass_kernel_spmd

    if trace:
        _ensure_ntff_hook()

    bf16 = ml_dtypes.bfloat16
    x = np.asarray(x, dtype=np.float32)
    W1 = np.asarray(W1, dtype=np.float32)
    b1 = np.asarray(b1, dtype=np.float32)
    W2 = np.asarray(W2, dtype=np.float32)
    b2 = np.asarray(b2, dtype=np.float32)

    xr = x.reshape(B, E, T, D)
    in_maps = []
    for e in range(E):
        xe = np.ascontiguousarray(xr[:, e]).reshape(NTOK, D)
        in_maps.append({
            "xT": xe.T.astype(bf16),                          # [D, NTOK]
            "W1": W1[e].astype(bf16),                         # [D, F]
            "b1": np.ascontiguousarray(b1[e].reshape(FS, 128).T),  # [128, FS]
            "W2": W2[e].astype(bf16),                         # [F, D]
            "b2": b2[e].reshape(1, D).copy(),                 # fp32
        })

    nc = _get_nc()
    res = run_bass_kernel_spmd(nc, in_maps, list(range(E)), trace=trace)
    LAST_RESULT = res

    out = np.empty((B, E * T, D), dtype=np.float32)
    for e in range(E):
        out[:, e * T:(e + 1) * T, :] = res.results[e]["y"].reshape(B, T, D)
    return out
